# revision 19
# baseline (speedup 1.0000x reference)
"""AttentionBlock3D (GroupNorm + 8-head softmax attention + out-proj) on 8 trn2 cores.

Sharding: one attention head per NeuronCore (tensor parallel on heads).
Each core:
  - loads the full x (256, 4096), computes GroupNorm(8 groups) locally
  - projects q/k/v for its head only (w_qkv row slices)
  - computes sim^T = k^T q in (j, i) layout (keys on partitions) so that
    exp(sim^T) can be consumed directly as the matmul moving operand of
    attn @ v without any transpose; the softmax denominator comes from an
    extra ones-column appended to v^T (flash-style unnormalized accumulate)
  - projects y^T_partial = out_h^T @ W_out_h^T, normalized by 1/den
Host: sums the 8 partial y^T, adds b_out, reshapes to (1, 256, 16, 16, 16).

Key trn2 tricks:
  - dim_head=32 => K=32 matmuls packed 4x into the 128x128 PE array via
    tile_position row-tiling (q/k replicated to 4 partition bands for free
    by replicating the projection weight columns host-side)
  - exp on ScalarE reads 3 PSUM banks (FD=1536) per instruction to amortize
    the per-op overhead; ScalarE is the bottleneck engine (~17M exps/core)
  - GroupNorm stats via bn_stats/bn_aggr + cross-partition group reduce via
    a block-diagonal ones matmul; rstd = exp(-0.5*ln(var+eps)) keeps ACT on
    the natural_log_exp table set (no table switch)
"""

from contextlib import ExitStack

import numpy as np

import concourse.bass as bass
import concourse.mybir as mybir
import concourse.tile as tile
from concourse import bacc
from concourse.bass_utils import run_bass_kernel_spmd

F32 = mybir.dt.float32
F32R = mybir.dt.float32r
AF = mybir.ActivationFunctionType
OP = mybir.AluOpType


def _r(ap):
    """Reinterpret an fp32 AP as float32r: single-pass PE matmul (1 cycle/row
    for moving dim >= 256, vs 4 for float32) at slightly reduced multiply
    precision. Storage layout is identical."""
    return ap.bitcast(F32R)

HEADS = 8
DH = 32
C = 256
N = 4096  # 16*16*16 tokens
NGROUPS = 8
GSIZE = C // NGROUPS  # 32 channels per group
EPS = 1e-5
SCALE = DH ** (-0.5)

IB = 512            # query block (matmul moving-operand free dim)
NIB = N // IB       # 8
JBLK = 128          # key block (PE partition dim)
NJB = N // JBLK     # 32
SIMG = 3            # j-blocks per PSUM sim tile / exp instruction (3 banks)

NCORES = 8


def _build_program():
    nc = bacc.Bacc(
        "TRN2", target_bir_lowering=False, debug=False, num_devices=NCORES
    )

    x_d = nc.declare_dram_parameter("x2d", [C, N], F32, isOutput=False)
    wq_d = nc.declare_dram_parameter("wq", [128, 2, 128], F32R, isOutput=False)
    wk_d = nc.declare_dram_parameter("wk", [128, 2, 128], F32R, isOutput=False)
    wv_d = nc.declare_dram_parameter("wv", [128, 2, DH], F32R, isOutput=False)
    wo_d = nc.declare_dram_parameter("wo", [DH, C], F32R, isOutput=False)
    gw_d = nc.declare_dram_parameter("gw", [128, 2], F32, isOutput=False)
    gb_d = nc.declare_dram_parameter("gb", [128, 2], F32, isOutput=False)
    bones_d = nc.declare_dram_parameter("bones", [128, 128], F32, isOutput=False)
    ident_d = nc.declare_dram_parameter("ident", [128, 128], F32R, isOutput=False)
    vones_d = nc.declare_dram_parameter("vones", [128, NJB], F32R, isOutput=False)
    yt_d = nc.declare_dram_parameter("yT", [N, C], F32, isOutput=True)

    with tile.TileContext(nc) as tc, ExitStack() as ctx:
        const = ctx.enter_context(tc.tile_pool(name="const", bufs=1))
        big = ctx.enter_context(tc.tile_pool(name="big", bufs=1))
        spool = ctx.enter_context(tc.tile_pool(name="stats", bufs=1))
        ppool = ctx.enter_context(tc.tile_pool(name="pbuf", bufs=4))
        ovt_pool = ctx.enter_context(tc.tile_pool(name="ovt", bufs=2))
        r_pool = ctx.enter_context(tc.tile_pool(name="rr", bufs=2))
        yt_pool = ctx.enter_context(tc.tile_pool(name="yt", bufs=2))
        ps_sim = ctx.enter_context(tc.tile_pool(name="ps_sim", bufs=2, space="PSUM"))
        ps_out = ctx.enter_context(tc.tile_pool(name="ps_out", bufs=1, space="PSUM"))
        ps_misc = ctx.enter_context(tc.tile_pool(name="ps_misc", bufs=1, space="PSUM"))

        # ---- constants / weights to SBUF ----
        wq_sb = const.tile([128, 2, 128], F32R)
        nc.sync.dma_start(out=wq_sb[:], in_=wq_d[:])
        wk_sb = const.tile([128, 2, 128], F32R)
        nc.sync.dma_start(out=wk_sb[:], in_=wk_d[:])
        wv_sb = const.tile([128, 2, DH], F32R)
        nc.sync.dma_start(out=wv_sb[:], in_=wv_d[:])
        wo_sb = const.tile([DH, C], F32R)
        nc.sync.dma_start(out=wo_sb[:], in_=wo_d[:])
        gw_sb = const.tile([128, 2], F32)
        nc.sync.dma_start(out=gw_sb[:], in_=gw_d[:])
        gb_sb = const.tile([128, 2], F32)
        nc.sync.dma_start(out=gb_sb[:], in_=gb_d[:])
        bones_sb = const.tile([128, 128], F32)
        nc.sync.dma_start(out=bones_sb[:], in_=bones_d[:])
        ident_sb = const.tile([128, 128], F32R)
        nc.sync.dma_start(out=ident_sb[:], in_=ident_d[:])
        eps_sb = const.tile([128, 1], F32)
        nc.vector.memset(eps_sb[:], EPS)

        # ---- load x (two 128-channel tiles), GroupNorm -> xn tiles ----
        xts = []
        xns = []
        for t in range(2):
            xt = big.tile([128, N], F32, tag=f"x{t}", name=f"x{t}")
            for cc in range(8):
                nc.sync.dma_start(
                    out=xt[:, cc * 512 : (cc + 1) * 512],
                    in_=x_d[t * 128 : (t + 1) * 128, cc * 512 : (cc + 1) * 512],
                )
            xts.append(xt)
            xn = big.tile([128, N], F32R, tag=f"xn{t}", name=f"xn{t}")
            xns.append(xn)

        gvars, gss = [], []
        for t in range(2):
            xt = xts[t]
            st = spool.tile([128, 8, 6], F32, tag=f"st{t}", name=f"st{t}")
            for cc in range(8):
                nc.vector.bn_stats(out=st[:, cc, :], in_=xt[:, cc * 512 : (cc + 1) * 512])
            mv = spool.tile([128, 2], F32, tag=f"mv{t}", name=f"mv{t}")
            nc.vector.bn_aggr(out=mv[:], in_=st[:])
            # per-channel [mean, E[x^2]]
            exm = spool.tile([128, 2], F32, tag=f"exm{t}", name=f"exm{t}")
            nc.vector.tensor_copy(out=exm[:, 0:1], in_=mv[:, 0:1])
            nc.vector.tensor_tensor(out=exm[:, 1:2], in0=mv[:, 0:1], in1=mv[:, 0:1], op=OP.mult)
            nc.vector.tensor_tensor(out=exm[:, 1:2], in0=exm[:, 1:2], in1=mv[:, 1:2], op=OP.add)
            # cross-partition group sum (broadcast back) via block-diagonal ones
            gps = ps_misc.tile([128, 2], F32, tag="misc", name=f"gps{t}")
            nc.tensor.matmul(gps[:], bones_sb[:], exm[:], start=True, stop=True)
            gs = spool.tile([128, 2], F32, tag=f"gs{t}", name=f"gs{t}")
            nc.vector.tensor_scalar_mul(out=gs[:], in0=gps[:], scalar1=1.0 / GSIZE)
            gvar = spool.tile([128, 1], F32, tag=f"gvar{t}", name=f"gvar{t}")
            nc.vector.tensor_tensor(out=gvar[:], in0=gs[:, 0:1], in1=gs[:, 0:1], op=OP.mult)
            nc.vector.tensor_tensor(out=gvar[:], in0=gs[:, 1:2], in1=gvar[:], op=OP.subtract)
            gvars.append(gvar)
            gss.append(gs)
        # rstd = exp(-0.5 * ln(var + eps)); both Lns then both Exps so the ACT
        # table set is loaded twice total, not four times
        lnvs = [spool.tile([128, 1], F32, tag=f"lnv{t}", name=f"lnv{t}") for t in range(2)]
        for t in range(2):
            nc.scalar.activation(out=lnvs[t][:], in_=gvars[t][:], func=AF.Ln, bias=eps_sb[:])
        rstds = [spool.tile([128, 1], F32, tag=f"rstd{t}", name=f"rstd{t}") for t in range(2)]
        for t in range(2):
            nc.scalar.activation(out=rstds[t][:], in_=lnvs[t][:], func=AF.Exp, scale=-0.5)
        abts = []
        for t in range(2):
            a_t = spool.tile([128, 1], F32, tag=f"a{t}", name=f"a{t}")
            nc.vector.tensor_tensor(out=a_t[:], in0=rstds[t][:], in1=gw_sb[:, t : t + 1], op=OP.mult)
            b_t = spool.tile([128, 1], F32, tag=f"b{t}", name=f"b{t}")
            nc.vector.tensor_tensor(out=b_t[:], in0=gss[t][:, 0:1], in1=a_t[:], op=OP.mult)
            nc.vector.tensor_tensor(out=b_t[:], in0=gb_sb[:, t : t + 1], in1=b_t[:], op=OP.subtract)
            abts.append((a_t, b_t))
        # xn = x * A + B, chunked so QKV matmuls can start on early chunks
        for cc in range(8):
            for t in range(2):
                a_t, b_t = abts[t]
                nc.vector.tensor_scalar(
                    out=xns[t][:, cc * 512 : (cc + 1) * 512],
                    in0=xts[t][:, cc * 512 : (cc + 1) * 512],
                    scalar1=a_t[:], scalar2=b_t[:],
                    op0=OP.mult, op1=OP.add,
                )

        # ---- QKV projections ----
        # q4/k4: (128, N) with the head's (32, N) q/k replicated on 4 partition
        # bands (weight columns were replicated host-side; M=128 matmul).
        q4 = big.tile([128, N], F32R, tag="q4", name="q4")
        k4 = big.tile([128, N], F32R, tag="k4", name="k4")
        for dst, wsb in ((q4, wq_sb), (k4, wk_sb)):
            for icnk in range(3):  # chunks of width 1536,1536,1024
                col0 = icnk * 3 * 512
                w = min(3 * 512, N - col0)
                qp = ps_sim.tile([128, 3 * 512], F32, tag="sim", name="qkp")
                for sub in range(w // 512):
                    c0 = col0 + sub * 512
                    nc.tensor.matmul(
                        qp[:, sub * 512 : sub * 512 + 512],
                        wsb[:, 0, :], xns[0][:, c0 : c0 + 512],
                        start=True, stop=False,
                    )
                    nc.tensor.matmul(
                        qp[:, sub * 512 : sub * 512 + 512],
                        wsb[:, 1, :], xns[1][:, c0 : c0 + 512],
                        start=False, stop=True,
                    )
                nc.vector.tensor_copy(out=dst[:, col0 : col0 + w], in_=qp[:, 0:w])

        # vT: (j, d) layout with a ones column at d=32 (softmax denominator)
        vt = big.tile([128, NJB, DH + 1], F32R, tag="vt", name="vt")
        nc.sync.dma_start(out=vt[:, :, DH : DH + 1], in_=vones_d.rearrange("p (n o) -> p n o", o=1))
        for vb in range(NJB // 4):
            vp = ps_sim.tile([128, 4, DH + 2], F32, tag="sim", name="vp")
            for s in range(4):
                jb = vb * 4 + s
                nc.tensor.matmul(
                    vp[:, s, 0:DH],
                    xns[0][:, jb * 128 : (jb + 1) * 128], wv_sb[:, 0, :],
                    start=True, stop=False,
                )
                nc.tensor.matmul(
                    vp[:, s, 0:DH],
                    xns[1][:, jb * 128 : (jb + 1) * 128], wv_sb[:, 1, :],
                    start=False, stop=True,
                )
            nc.vector.tensor_copy(
                out=vt[:, vb * 4 : (vb + 1) * 4, 0:DH], in_=vp[:, :, 0:DH]
            )

        # ---- attention main loop ----
        groups = [list(range(g, min(g + SIMG, NJB))) for g in range(0, NJB, SIMG)]
        for ib in range(NIB):
            icol = ib * IB
            outp = ps_out.tile([128, IB], F32, tag="outp", name="outp")
            for jbs in groups:
                simp = ps_sim.tile([128, SIMG * IB], F32, tag="sim", name="simp")
                for s, jb in enumerate(jbs):
                    band = jb % 4
                    # sim^T(j_block, i_block) = k_slice^T q_slice ; K=32 row-packed
                    nc.tensor.matmul(
                        simp[:, s * IB : (s + 1) * IB],
                        k4[band * 32 : (band + 1) * 32, jb * 128 : (jb + 1) * 128],
                        q4[band * 32 : (band + 1) * 32, icol : icol + IB],
                        start=True, stop=True,
                        tile_position=(band * 32, 0),
                    )
                w = len(jbs) * IB
                psb = ppool.tile([128, SIMG * IB], F32R, tag="p", name="psb")
                nc.scalar.activation(
                    out=psb[:, 0:w], in_=simp[:, 0:w], func=AF.Exp, scale=SCALE
                )
                for s, jb in enumerate(jbs):
                    # out'(d+1, i) += vT(jb)^T @ P(jb)   (row 32 = denominator)
                    nc.tensor.matmul(
                        outp[0 : DH + 1, :],
                        vt[:, jb, :],
                        psb[:, s * IB : (s + 1) * IB],
                        start=(jb == 0), stop=(jb == NJB - 1),
                    )
            ovt = ovt_pool.tile([DH + 2, IB], F32R, tag="ovt", name="ovt")
            nc.vector.tensor_copy(out=ovt[0 : DH + 1, :], in_=outp[0 : DH + 1, :])
            for cch in range(IB // 128):
                csl = slice(cch * 128, (cch + 1) * 128)
                trp = ps_misc.tile([128, DH + 2], F32R, tag="misc", name="trp")
                nc.tensor.transpose(trp[:], ovt[0 : DH + 2, csl], ident_sb[0 : DH + 2, 0 : DH + 2])
                rr = r_pool.tile([128, 1], F32, tag="rr", name="rr")
                nc.vector.reciprocal(out=rr[:], in_=trp[:, DH : DH + 1])
                ytp = ps_misc.tile([128, C], F32, tag="misc", name="ytp")
                nc.tensor.matmul(
                    ytp[:], ovt[0:DH, csl], wo_sb[:], start=True, stop=True
                )
                yts = yt_pool.tile([128, C], F32, tag="yt", name="yts")
                nc.vector.tensor_scalar_mul(out=yts[:], in0=ytp[:], scalar1=rr[:])
                nc.sync.dma_start(
                    out=yt_d[icol + cch * 128 : icol + (cch + 1) * 128, :], in_=yts[:]
                )

    nc.compile()
    return nc


_CACHE: dict = {}


def _get_program():
    if "nc" not in _CACHE:
        _CACHE["nc"] = _build_program()
    return _CACHE["nc"]


def _make_in_maps(x, gn_weight, gn_bias, w_qkv, w_out):
    x2d = np.ascontiguousarray(x.reshape(C, N), dtype=np.float32)
    gw = np.ascontiguousarray(gn_weight.reshape(2, 128).T, dtype=np.float32)
    gb = np.ascontiguousarray(gn_bias.reshape(2, 128).T, dtype=np.float32)
    bones = np.zeros((128, 128), dtype=np.float32)
    for g in range(128 // GSIZE):
        bones[g * GSIZE : (g + 1) * GSIZE, g * GSIZE : (g + 1) * GSIZE] = 1.0
    ident = np.eye(128, dtype=np.float32)

    in_maps = []
    for h in range(NCORES):
        rq = slice(h * DH, (h + 1) * DH)
        wq = w_qkv[rq, :]                      # (32, 256)
        wk = w_qkv[HEADS * DH + h * DH : HEADS * DH + (h + 1) * DH, :]
        wv = w_qkv[2 * HEADS * DH + h * DH : 2 * HEADS * DH + (h + 1) * DH, :]
        # (128, 2, 128): [channel_in_tile, c_tile, 4x-replicated head dim]
        wq4 = np.tile(wq.T, (1, 4)).reshape(2, 128, 128).transpose(1, 0, 2)
        wk4 = np.tile(wk.T, (1, 4)).reshape(2, 128, 128).transpose(1, 0, 2)
        wvt = wv.T.reshape(2, 128, DH).transpose(1, 0, 2)  # (128, 2, 32)
        wo = w_out[:, rq].T                    # (32, 256)
        in_maps.append(
            {
                "x2d": x2d,
                "wq": np.ascontiguousarray(wq4, dtype=np.float32),
                "wk": np.ascontiguousarray(wk4, dtype=np.float32),
                "wv": np.ascontiguousarray(wvt, dtype=np.float32),
                "wo": np.ascontiguousarray(wo, dtype=np.float32),
                "gw": gw,
                "gb": gb,
                "bones": bones,
                "ident": ident,
                "vones": np.ones((128, NJB), dtype=np.float32),
            }
        )
    return in_maps


def run_sharded(x, gn_weight, gn_bias, w_qkv, w_out, b_out, **run_kwargs):
    """Run the SPMD kernel; returns (full_output, BassKernelResults)."""
    nc = _get_program()
    in_maps = _make_in_maps(
        np.asarray(x), np.asarray(gn_weight), np.asarray(gn_bias),
        np.asarray(w_qkv), np.asarray(w_out),
    )
    res = run_bass_kernel_spmd(nc, in_maps, core_ids=list(range(NCORES)), **run_kwargs)
    yt = np.zeros((N, C), dtype=np.float64)
    for r in res.results:
        yt += np.asarray(r["yT"], dtype=np.float64)
    y = yt.T + np.asarray(b_out, dtype=np.float64)[:, None]
    out = y.astype(np.float32).reshape(1, C, 16, 16, 16)
    return out, res


def kernel(x, gn_weight, gn_bias, w_qkv, w_out, b_out):
    out, _ = run_sharded(x, gn_weight, gn_bias, w_qkv, w_out, b_out)
    return out
